# revision 7
# baseline (speedup 1.0000x reference)
"""LMU kernel for Trainium2, 8-core data-parallel.

Math (per batch b, with x[b] in [D, L] layout):
  u[b]    = relu(W_u @ x[b] + b_u)                              [1, L]
  m[b]    = H @ Toep(u[b])        (causal conv via Toeplitz)    [D, L]
  h[b]    = relu(W_h[:, :D] @ m[b] + W_h[:, D:] @ x[b] + b_h)   [D, L]
  y[b]    = BN(conv_w @ h[b] + conv_b)                          [D, L]

Device-side folds (host precomputes, O(params) only):
  F      = (W_h[:, :D] @ H).T, row-flipped  -> single K=128 contraction
           against the (flipped) Toeplitz of u
  C'     = (inv * conv_w).T, bias' = (conv_b - mean) * inv + beta   (BN fold)

All tensors live as float32r end-to-end (bit-identical to f32, streams
1 col/cycle on the PE) so DMA deposits matmul-ready tiles with zero
staging casts.  Weight matrices are staged j-major (output-chunk major)
on a single deadline-ordered DMA queue so the first block's matmuls
never wait on staging.  Batch dim sharded 8 ways; params replicated.
"""

import os
import numpy as np

import concourse.bass as bass
import concourse.mybir as mybir
from concourse import bacc
from concourse.tile import TileContext
from concourse.bass_utils import run_bass_kernel_spmd

B, D, L = 256, 768, 128
NCORES = 8
BPC = B // NCORES          # batches per core
NB = 4                     # batches per column block
NCB = BPC // NB            # column blocks per core
NCOL = NB * L              # 512 columns per block
KC = D // 128              # 6 chunks of 128 over the D dim
THETA = 128.0
BN_EPS = 1e-5

TRACE = False
LAST_EXEC_NS = None

_H_CACHE = None
_NC_CACHE = None


def _impulse_response():
    """Replicates the reference's H = impulse response [D, L], on CPU."""
    global _H_CACHE
    if _H_CACHE is not None:
        return _H_CACHE
    import jax
    import jax.numpy as jnp
    from jax.scipy.linalg import expm

    cpu = jax.devices("cpu")[0]
    with jax.default_device(cpu):
        Q = np.arange(D, dtype=np.float32)
        R = ((2.0 * Q + 1.0) / THETA)[:, None]
        i, j = np.meshgrid(Q, Q, indexing="ij")
        A = (np.where(i < j, -1.0, (-1.0) ** (i - j + 1)).astype(np.float32)) * R
        Bm = (((-1.0) ** Q)[:, None]).astype(np.float32) * R
        Maug = np.zeros((D + 1, D + 1), dtype=np.float32)
        Maug[:D, :D] = A
        Maug[:D, D:] = Bm
        E = expm(jnp.asarray(Maug))
        Ad = E[:D, :D]
        Bd = E[:D, D:]

        def step(Apow, _):
            return Ad @ Apow, (Apow @ Bd)[:, 0]

        _, H = jax.lax.scan(step, jnp.eye(D, dtype=jnp.float32), None, length=L)
        _H_CACHE = np.asarray(H).T.astype(np.float32)  # [D, L]
    return _H_CACHE


def _build_nc():
    """Builds the (static) 8-core SPMD Bass program."""
    f32 = mybir.dt.float32
    f32r = mybir.dt.float32r
    nc = bacc.Bacc("TRN2", target_bir_lowering=False, debug=False, num_devices=NCORES)

    x_d = nc.dram_tensor("x", [BPC, D, L], f32r, kind="ExternalInput").ap()
    whxT_d = nc.dram_tensor("whxT", [D, D], f32r, kind="ExternalInput").ap()
    ct_d = nc.dram_tensor("ct", [D, D], f32r, kind="ExternalInput").ap()
    f_d = nc.dram_tensor("fmat", [L, D], f32r, kind="ExternalInput").ap()
    wu_d = nc.dram_tensor("wu", [128, KC], f32r, kind="ExternalInput").ap()
    vecs_d = nc.dram_tensor("vecs", [D, 3], f32, kind="ExternalInput").ap()
    out_d = nc.dram_tensor("out", [BPC, D, L], f32r, kind="ExternalOutput").ap()
    upad_d = nc.dram_tensor("upad", [BPC * 2 * L], f32r).ap()  # internal scratch

    XSTR_B, XSTR_D = D * L, L  # element strides of x / out in DRAM
    Relu = mybir.ActivationFunctionType.Relu

    with TileContext(nc) as tc:
        with (
            tc.tile_pool(name="const", bufs=1) as const,
            tc.tile_pool(name="xpool", bufs=18) as xpool,
            tc.tile_pool(name="hpool", bufs=12) as hpool,
            tc.tile_pool(name="tpool", bufs=3) as tpool,
            tc.tile_pool(name="opool", bufs=6) as opool,
            tc.tile_pool(name="upool", bufs=2) as upool,
            tc.tile_pool(name="pu", bufs=2, space="PSUM") as pu,
            tc.tile_pool(name="p3", bufs=3, space="PSUM") as p3,
            tc.tile_pool(name="p4", bufs=3, space="PSUM") as p4,
        ):
            # ---- constant tiles (DMA'd directly, no staging casts) ----
            whx_r = const.tile([128, KC, D], f32r)   # [d part | i_chunk | d' col]
            ct_r = const.tile([128, KC, D], f32r)    # [dh part | i_chunk | o col]
            f_r = const.tile([128, D], f32r)         # [t' part | d]
            wu_r = const.tile([128, KC], f32r)
            vecs_sb = const.tile([128, KC, 3], f32)  # b_h, bias', b_u
            zt = const.tile([128, 2 * BPC], f32)

            # small params on scalar (HWDGE, low latency): needed by u(0)
            nc.scalar.dma_start(out=wu_r[:], in_=wu_d)
            nc.scalar.dma_start(
                out=vecs_sb[:],
                in_=bass.AP(tensor=vecs_d.tensor, offset=0,
                            ap=[[3, 128], [384, KC], [1, 3]]),
            )
            # zero the upad scratch (pad halves stay zero forever)
            nc.vector.memset(zt[:], 0.0)
            nc.gpsimd.dma_start(
                out=bass.AP(tensor=upad_d.tensor, offset=0,
                            ap=[[1, BPC * 2 * L]]),
                in_=zt[:],
            )

            def load_x(cb):
                """DMA x tiles for column block cb straight into f32r tiles.

                Chunks alternate sync/scalar so a block lands in ~half the
                time and neither queue saturates.
                """
                b0 = cb * NB
                xr = []
                for i in range(KC):
                    xt = xpool.tile([128, NCOL], f32r, tag="xt")
                    eng = nc.sync if i % 2 == 0 else nc.scalar
                    eng.dma_start(
                        out=xt[:],
                        in_=bass.AP(
                            tensor=x_d.tensor,
                            offset=b0 * XSTR_B + i * 128 * XSTR_D,
                            ap=[[XSTR_D, 128], [XSTR_B, NB], [1, L]],
                        ),
                    )
                    xr.append(xt)
                return xr

            def stage_w_chunk(dram, dst, j, eng):
                """Stage output-chunk j of a [D, D] weight (all K rows)."""
                eng.dma_start(
                    out=dst[:, :, j * 128:(j + 1) * 128],
                    in_=bass.AP(
                        tensor=dram.tensor,
                        offset=j * 128,
                        ap=[[D, 128], [128 * D, KC], [1, 128]],
                    ),
                )

            def compute_u(cb, xr, chain_eng):
                """u = relu(W_u @ x + b_u) -> upad scratch -> Toeplitz tile."""
                psu = pu.tile([1, NCOL], f32, tag="pu")
                for i in range(KC):
                    nc.tensor.matmul(psu[:], wu_r[:, i:i + 1], xr[i][:],
                                     start=(i == 0), stop=(i == KC - 1))
                u_sb = upool.tile([1, NCOL], f32r, tag="u")
                nc.scalar.activation(u_sb[:], psu[:], Relu,
                                     bias=vecs_sb[0:1, 0, 2:3])
                chain_eng.dma_start(
                    out=bass.AP(tensor=upad_d.tensor,
                                offset=cb * NB * 2 * L + L,
                                ap=[[2 * L, NB], [1, L]]),
                    in_=u_sb[:],
                )
                t_r = tpool.tile([128, NCOL], f32r, tag="tr")
                chain_eng.dma_start(
                    out=t_r[:],
                    in_=bass.AP(tensor=upad_d.tensor,
                                offset=cb * NB * 2 * L + 1,
                                ap=[[1, 128], [2 * L, NB], [1, L]]),
                )
                return t_r

            def step3(cb, xr, t_r, js, hs):
                for j in js:
                    ps3 = p3.tile([128, NCOL], f32, tag="ps3")
                    for i in range(KC):
                        nc.tensor.matmul(ps3[:], whx_r[:, i, j * 128:(j + 1) * 128],
                                         xr[i][:], start=(i == 0), stop=False)
                    nc.tensor.matmul(ps3[:], f_r[:, j * 128:(j + 1) * 128], t_r[:],
                                     start=False, stop=True)
                    hj = hpool.tile([128, NCOL], f32r, tag="h")
                    nc.scalar.activation(hj[:], ps3[:], Relu,
                                         bias=vecs_sb[:, j, 0:1])
                    hs.append(hj)

            def step4(cb, hs):
                b0 = cb * NB
                for j in range(KC):
                    ps4 = p4.tile([128, NCOL], f32, tag="ps4")
                    for i in range(KC):
                        nc.tensor.matmul(ps4[:], ct_r[:, i, j * 128:(j + 1) * 128],
                                         hs[i][:], start=(i == 0), stop=(i == KC - 1))
                    oj = opool.tile([128, NCOL], f32r, tag="o")
                    nc.vector.tensor_scalar_add(oj[:], ps4[:], vecs_sb[:, j, 1:2])
                    oeng = nc.sync if j % 2 == 0 else nc.scalar
                    oeng.dma_start(
                        out=bass.AP(
                            tensor=out_d.tensor,
                            offset=b0 * XSTR_B + j * 128 * XSTR_D,
                            ap=[[XSTR_D, 128], [XSTR_B, NB], [1, L]],
                        ),
                        in_=oj[:],
                    )

            # ---- prologue: x(0) split across sync+scalar, whx/f on sync
            # behind it (needed from ~18us), ct on the otherwise-idle
            # gpsimd queue (needed from ~28us), x(1) right behind x(0) so
            # u(1) can fill the PE while block 0's Toeplitz round-trip is
            # in flight.
            xr = {0: load_x(0)}
            t = {0: compute_u(0, xr[0], nc.scalar)}
            for j in range(KC):
                stage_w_chunk(whxT_d, whx_r, j, nc.sync)
            nc.sync.dma_start(out=f_r[:], in_=f_d)
            xr[1] = load_x(1)
            for j in range(KC):
                stage_w_chunk(ct_d, ct_r, j, nc.gpsimd)

            # ---- software-pipelined main loop: x stays two blocks ahead;
            # the u->Toeplitz chain for block cb+1 is issued at the top of
            # block cb (a full block of step3/step4 hides its ~8us DMA
            # round-trip latency).
            for cb in range(NCB):
                if cb + 2 < NCB:
                    xr[cb + 2] = load_x(cb + 2)
                if cb >= 1 and cb + 1 < NCB:
                    t[cb + 1] = compute_u(cb + 1, xr[cb + 1],
                                          nc.scalar if cb == 1 else nc.gpsimd)
                hs = []
                step3(cb, xr[cb], t[cb], [0, 1, 2], hs)
                if cb == 0:
                    t[1] = compute_u(1, xr[1], nc.scalar)
                step3(cb, xr[cb], t[cb], [3, 4, 5], hs)
                step4(cb, hs)

    if not nc.is_finalized():
        nc.finalize()
    return nc


def _get_nc():
    global _NC_CACHE
    if _NC_CACHE is None:
        _NC_CACHE = _build_nc()
    return _NC_CACHE


def _ensure_ntff_hook():
    """Register the NTFF profile hook if the deployment lacks antenv.axon_hooks."""
    import sys
    import types
    try:
        from antenv.axon_hooks import get_axon_ntff_profile_hook  # noqa: F401
        return
    except ImportError:
        pass
    try:
        from trn_agent_boot.trn_boot import _ntff_profile_via_ctypes
        hook = _ntff_profile_via_ctypes("/opt/axon/libaxon_pjrt.so")
        mod = types.ModuleType("antenv.axon_hooks")
        mod.get_axon_ntff_profile_hook = lambda: hook
        mod.set_axon_ntff_profile_hook = lambda h: None
        import antenv
        sys.modules["antenv.axon_hooks"] = mod
        antenv.axon_hooks = mod
    except Exception:
        pass


def kernel(x, W_u, b_u, W_h, b_h, conv_w, conv_b, bn_gamma, bn_beta, bn_mean,
           bn_var):
    global LAST_EXEC_NS
    x = np.ascontiguousarray(np.asarray(x, dtype=np.float32))
    W_u = np.asarray(W_u, dtype=np.float64)
    b_u = np.asarray(b_u, dtype=np.float64)
    W_h = np.asarray(W_h, dtype=np.float64)
    b_h = np.asarray(b_h, dtype=np.float64)
    conv_w = np.asarray(conv_w, dtype=np.float64)
    conv_b = np.asarray(conv_b, dtype=np.float64)
    bn_gamma = np.asarray(bn_gamma, dtype=np.float64)
    bn_beta = np.asarray(bn_beta, dtype=np.float64)
    bn_mean = np.asarray(bn_mean, dtype=np.float64)
    bn_var = np.asarray(bn_var, dtype=np.float64)
    assert x.shape == (B, D, L)

    H = _impulse_response().astype(np.float64)  # [D, L]

    # host folds (O(params) only)
    F = (W_h[:, :D] @ H).T[::-1, :]                      # [L, D], row-flipped
    whxT = np.ascontiguousarray(W_h[:, D:].T)            # [D(d), D(d')]
    inv = bn_gamma / np.sqrt(bn_var + BN_EPS)
    ct = np.ascontiguousarray((conv_w[:, :, 0] * inv[:, None]).T)  # [dh, o]
    bias2 = (conv_b - bn_mean) * inv + bn_beta
    wu = np.ascontiguousarray(W_u[0].reshape(KC, 128).T)  # [128, KC]
    vecs = np.stack([b_h, bias2, np.full(D, b_u[0])], axis=1)  # [D, 3]

    nc = _get_nc()
    shared = {
        "whxT": whxT.astype(np.float32),
        "ct": ct.astype(np.float32),
        "fmat": np.ascontiguousarray(F).astype(np.float32),
        "wu": wu.astype(np.float32),
        "vecs": vecs.astype(np.float32),
    }
    in_maps = []
    for c in range(NCORES):
        m = dict(shared)
        m["x"] = x[c * BPC:(c + 1) * BPC]
        in_maps.append(m)

    if TRACE:
        _ensure_ntff_hook()
    res = run_bass_kernel_spmd(nc, in_maps, list(range(NCORES)), trace=TRACE)
    LAST_EXEC_NS = res.exec_time_ns
    out = np.concatenate([res.results[c]["out"] for c in range(NCORES)], axis=0)
    return out


# revision 10
# speedup vs baseline: 1.0292x; 1.0292x over previous
"""LMU kernel for Trainium2, 8-core data-parallel.

Math (per batch b, with x[b] in [D, L] layout):
  u[b]    = relu(W_u @ x[b] + b_u)                              [1, L]
  m[b]    = H @ Toep(u[b])        (causal conv via Toeplitz)    [D, L]
  h[b]    = relu(W_h[:, :D] @ m[b] + W_h[:, D:] @ x[b] + b_h)   [D, L]
  y[b]    = BN(conv_w @ h[b] + conv_b)                          [D, L]

Device-side folds (host precomputes, O(params) only):
  F      = (W_h[:, :D] @ H).T, row-flipped  -> single K=128 contraction
           against the (flipped) Toeplitz of u
  C'     = (inv * conv_w).T, bias' = (conv_b - mean) * inv + beta   (BN fold)

All tensors live as float32r end-to-end (bit-identical to f32, streams
1 col/cycle on the PE) so DMA deposits matmul-ready tiles with zero
staging casts.  Weight matrices are staged j-major (output-chunk major)
on a single deadline-ordered DMA queue so the first block's matmuls
never wait on staging.  Batch dim sharded 8 ways; params replicated.
"""

import os
import numpy as np

import concourse.bass as bass
import concourse.mybir as mybir
from concourse import bacc
from concourse.tile import TileContext
from concourse.bass_utils import run_bass_kernel_spmd

B, D, L = 256, 768, 128
NCORES = 8
BPC = B // NCORES          # batches per core
NB = 4                     # batches per column block
NCB = BPC // NB            # column blocks per core
NCOL = NB * L              # 512 columns per block
KC = D // 128              # 6 chunks of 128 over the D dim
THETA = 128.0
BN_EPS = 1e-5

TRACE = False
LAST_EXEC_NS = None

_H_CACHE = None
_NC_CACHE = None


def _impulse_response():
    """Replicates the reference's H = impulse response [D, L], on CPU."""
    global _H_CACHE
    if _H_CACHE is not None:
        return _H_CACHE
    import jax
    import jax.numpy as jnp
    from jax.scipy.linalg import expm

    cpu = jax.devices("cpu")[0]
    with jax.default_device(cpu):
        Q = np.arange(D, dtype=np.float32)
        R = ((2.0 * Q + 1.0) / THETA)[:, None]
        i, j = np.meshgrid(Q, Q, indexing="ij")
        A = (np.where(i < j, -1.0, (-1.0) ** (i - j + 1)).astype(np.float32)) * R
        Bm = (((-1.0) ** Q)[:, None]).astype(np.float32) * R
        Maug = np.zeros((D + 1, D + 1), dtype=np.float32)
        Maug[:D, :D] = A
        Maug[:D, D:] = Bm
        E = expm(jnp.asarray(Maug))
        Ad = E[:D, :D]
        Bd = E[:D, D:]

        def step(Apow, _):
            return Ad @ Apow, (Apow @ Bd)[:, 0]

        _, H = jax.lax.scan(step, jnp.eye(D, dtype=jnp.float32), None, length=L)
        _H_CACHE = np.asarray(H).T.astype(np.float32)  # [D, L]
    return _H_CACHE


def _build_nc():
    """Builds the (static) 8-core SPMD Bass program."""
    f32 = mybir.dt.float32
    f32r = mybir.dt.float32r
    nc = bacc.Bacc("TRN2", target_bir_lowering=False, debug=False, num_devices=NCORES)

    x_d = nc.dram_tensor("x", [BPC, D, L], f32r, kind="ExternalInput").ap()
    whxT_d = nc.dram_tensor("whxT", [D, D], f32r, kind="ExternalInput").ap()
    ct_d = nc.dram_tensor("ct", [D, D], f32r, kind="ExternalInput").ap()
    f_d = nc.dram_tensor("fmat", [L, D], f32r, kind="ExternalInput").ap()
    wu_d = nc.dram_tensor("wu", [128, KC], f32r, kind="ExternalInput").ap()
    vecs_d = nc.dram_tensor("vecs", [D, 3], f32, kind="ExternalInput").ap()
    out_d = nc.dram_tensor("out", [BPC, D, L], f32r, kind="ExternalOutput").ap()
    upad_d = nc.dram_tensor("upad", [BPC * 2 * L], f32r).ap()  # internal scratch

    XSTR_B, XSTR_D = D * L, L  # element strides of x / out in DRAM
    Relu = mybir.ActivationFunctionType.Relu

    with TileContext(nc) as tc:
        with (
            tc.tile_pool(name="const", bufs=1) as const,
            tc.tile_pool(name="xpool", bufs=18) as xpool,
            tc.tile_pool(name="hpool", bufs=12) as hpool,
            tc.tile_pool(name="tpool", bufs=3) as tpool,
            tc.tile_pool(name="opool", bufs=6) as opool,
            tc.tile_pool(name="upool", bufs=2) as upool,
            tc.tile_pool(name="pu", bufs=2, space="PSUM") as pu,
            tc.tile_pool(name="p3", bufs=3, space="PSUM") as p3,
            tc.tile_pool(name="p4", bufs=3, space="PSUM") as p4,
        ):
            # ---- constant tiles (DMA'd directly, no staging casts) ----
            whx_r = const.tile([128, KC, D], f32r)   # [d part | i_chunk | d' col]
            ct_r = const.tile([128, KC, D], f32r)    # [dh part | i_chunk | o col]
            f_r = const.tile([128, D], f32r)         # [t' part | d]
            wu_r = const.tile([128, KC], f32r)
            vecs_sb = const.tile([128, KC, 3], f32)  # b_h, bias', b_u
            zt = const.tile([128, 2 * BPC], f32)

            # small params on scalar (HWDGE, low latency): needed by u(0)
            nc.scalar.dma_start(out=wu_r[:], in_=wu_d)
            nc.scalar.dma_start(
                out=vecs_sb[:],
                in_=bass.AP(tensor=vecs_d.tensor, offset=0,
                            ap=[[3, 128], [384, KC], [1, 3]]),
            )
            # zero the upad scratch (pad halves stay zero forever)
            nc.vector.memset(zt[:], 0.0)
            nc.gpsimd.dma_start(
                out=bass.AP(tensor=upad_d.tensor, offset=0,
                            ap=[[1, BPC * 2 * L]]),
                in_=zt[:],
            )

            def load_x(cb):
                """DMA x tiles for column block cb straight into f32r tiles.

                Chunks alternate sync/scalar so a block lands in ~half the
                time and neither queue saturates.
                """
                b0 = cb * NB
                xr = []
                for i in range(KC):
                    xt = xpool.tile([128, NCOL], f32r, tag="xt")
                    eng = nc.sync
                    eng.dma_start(
                        out=xt[:],
                        in_=bass.AP(
                            tensor=x_d.tensor,
                            offset=b0 * XSTR_B + i * 128 * XSTR_D,
                            ap=[[XSTR_D, 128], [XSTR_B, NB], [1, L]],
                        ),
                    )
                    xr.append(xt)
                return xr

            def stage_w_chunk(dram, dst, j, eng):
                """Stage output-chunk j of a [D, D] weight (all K rows)."""
                eng.dma_start(
                    out=dst[:, :, j * 128:(j + 1) * 128],
                    in_=bass.AP(
                        tensor=dram.tensor,
                        offset=j * 128,
                        ap=[[D, 128], [128 * D, KC], [1, 128]],
                    ),
                )

            def compute_u(cb, xr, chain_eng):
                """u = relu(W_u @ x + b_u) -> upad scratch -> Toeplitz tile."""
                psu = pu.tile([1, NCOL], f32, tag="pu")
                for i in range(KC):
                    nc.tensor.matmul(psu[:], wu_r[:, i:i + 1], xr[i][:],
                                     start=(i == 0), stop=(i == KC - 1))
                u_sb = upool.tile([1, NCOL], f32r, tag="u")
                nc.scalar.activation(u_sb[:], psu[:], Relu,
                                     bias=vecs_sb[0:1, 0, 2:3])
                chain_eng.dma_start(
                    out=bass.AP(tensor=upad_d.tensor,
                                offset=cb * NB * 2 * L + L,
                                ap=[[2 * L, NB], [1, L]]),
                    in_=u_sb[:],
                )
                t_r = tpool.tile([128, NCOL], f32r, tag="tr")
                chain_eng.dma_start(
                    out=t_r[:],
                    in_=bass.AP(tensor=upad_d.tensor,
                                offset=cb * NB * 2 * L + 1,
                                ap=[[1, 128], [2 * L, NB], [1, L]]),
                )
                return t_r

            def step3(cb, xr, t_r, js, hs):
                for j in js:
                    ps3 = p3.tile([128, NCOL], f32, tag="ps3")
                    for i in range(KC):
                        nc.tensor.matmul(ps3[:], whx_r[:, i, j * 128:(j + 1) * 128],
                                         xr[i][:], start=(i == 0), stop=False)
                    nc.tensor.matmul(ps3[:], f_r[:, j * 128:(j + 1) * 128], t_r[:],
                                     start=False, stop=True)
                    hj = hpool.tile([128, NCOL], f32r, tag="h")
                    nc.scalar.activation(hj[:], ps3[:], Relu,
                                         bias=vecs_sb[:, j, 0:1])
                    hs.append(hj)

            def step4(cb, hs):
                b0 = cb * NB
                for j in range(KC):
                    ps4 = p4.tile([128, NCOL], f32, tag="ps4")
                    for i in range(KC):
                        nc.tensor.matmul(ps4[:], ct_r[:, i, j * 128:(j + 1) * 128],
                                         hs[i][:], start=(i == 0), stop=(i == KC - 1))
                    oj = opool.tile([128, NCOL], f32r, tag="o")
                    nc.vector.tensor_scalar_add(oj[:], ps4[:], vecs_sb[:, j, 1:2])
                    oeng = nc.scalar
                    oeng.dma_start(
                        out=bass.AP(
                            tensor=out_d.tensor,
                            offset=b0 * XSTR_B + j * 128 * XSTR_D,
                            ap=[[XSTR_D, 128], [XSTR_B, NB], [1, L]],
                        ),
                        in_=oj[:],
                    )

            # ---- prologue: ONE deadline-ordered staging queue (sync).
            # Concurrent queues share the same 16 SDMA engines, so a second
            # queue only delays whichever transfer is needed first — strict
            # need-order on one queue wins.  Deadlines (PE time): x(0) asap,
            # whx j0-2 ~17us (first step3 matmuls), f ~22 (first Toeplitz
            # matmul), x(1) ~22 (u(1) fills the Toeplitz-wait), whx j3-5
            # ~26, ct ~30 (step4(0)), x(2) ~36, x(3+) one block ahead.
            xr = {0: load_x(0)}
            t = {0: compute_u(0, xr[0], nc.scalar)}
            for j in range(3):
                stage_w_chunk(whxT_d, whx_r, j, nc.sync)
            nc.sync.dma_start(out=f_r[:], in_=f_d)
            xr[1] = load_x(1)
            for j in range(3, KC):
                stage_w_chunk(whxT_d, whx_r, j, nc.sync)
            for j in range(KC):
                stage_w_chunk(ct_d, ct_r, j, nc.sync)

            # ---- software-pipelined main loop: x stays two blocks ahead;
            # the u->Toeplitz chain for block cb+1 is issued at the top of
            # block cb (a full block of step3/step4 hides its ~8us DMA
            # round-trip latency).
            for cb in range(NCB):
                if cb + 2 < NCB:
                    xr[cb + 2] = load_x(cb + 2)
                if cb >= 1 and cb + 1 < NCB:
                    t[cb + 1] = compute_u(cb + 1, xr[cb + 1],
                                          nc.scalar if cb == 1 else nc.gpsimd)
                hs = []
                step3(cb, xr[cb], t[cb], [0, 1, 2], hs)
                if cb == 0:
                    t[1] = compute_u(1, xr[1], nc.scalar)
                step3(cb, xr[cb], t[cb], [3, 4, 5], hs)
                step4(cb, hs)

    if not nc.is_finalized():
        nc.finalize()
    return nc


def _get_nc():
    global _NC_CACHE
    if _NC_CACHE is None:
        _NC_CACHE = _build_nc()
    return _NC_CACHE


def _ensure_ntff_hook():
    """Register the NTFF profile hook if the deployment lacks antenv.axon_hooks."""
    import sys
    import types
    try:
        from antenv.axon_hooks import get_axon_ntff_profile_hook  # noqa: F401
        return
    except ImportError:
        pass
    try:
        from trn_agent_boot.trn_boot import _ntff_profile_via_ctypes
        hook = _ntff_profile_via_ctypes("/opt/axon/libaxon_pjrt.so")
        mod = types.ModuleType("antenv.axon_hooks")
        mod.get_axon_ntff_profile_hook = lambda: hook
        mod.set_axon_ntff_profile_hook = lambda h: None
        import antenv
        sys.modules["antenv.axon_hooks"] = mod
        antenv.axon_hooks = mod
    except Exception:
        pass


def kernel(x, W_u, b_u, W_h, b_h, conv_w, conv_b, bn_gamma, bn_beta, bn_mean,
           bn_var):
    global LAST_EXEC_NS
    x = np.ascontiguousarray(np.asarray(x, dtype=np.float32))
    W_u = np.asarray(W_u, dtype=np.float64)
    b_u = np.asarray(b_u, dtype=np.float64)
    W_h = np.asarray(W_h, dtype=np.float64)
    b_h = np.asarray(b_h, dtype=np.float64)
    conv_w = np.asarray(conv_w, dtype=np.float64)
    conv_b = np.asarray(conv_b, dtype=np.float64)
    bn_gamma = np.asarray(bn_gamma, dtype=np.float64)
    bn_beta = np.asarray(bn_beta, dtype=np.float64)
    bn_mean = np.asarray(bn_mean, dtype=np.float64)
    bn_var = np.asarray(bn_var, dtype=np.float64)
    assert x.shape == (B, D, L)

    H = _impulse_response().astype(np.float64)  # [D, L]

    # host folds (O(params) only)
    F = (W_h[:, :D] @ H).T[::-1, :]                      # [L, D], row-flipped
    whxT = np.ascontiguousarray(W_h[:, D:].T)            # [D(d), D(d')]
    inv = bn_gamma / np.sqrt(bn_var + BN_EPS)
    ct = np.ascontiguousarray((conv_w[:, :, 0] * inv[:, None]).T)  # [dh, o]
    bias2 = (conv_b - bn_mean) * inv + bn_beta
    wu = np.ascontiguousarray(W_u[0].reshape(KC, 128).T)  # [128, KC]
    vecs = np.stack([b_h, bias2, np.full(D, b_u[0])], axis=1)  # [D, 3]

    nc = _get_nc()
    shared = {
        "whxT": whxT.astype(np.float32),
        "ct": ct.astype(np.float32),
        "fmat": np.ascontiguousarray(F).astype(np.float32),
        "wu": wu.astype(np.float32),
        "vecs": vecs.astype(np.float32),
    }
    in_maps = []
    for c in range(NCORES):
        m = dict(shared)
        m["x"] = x[c * BPC:(c + 1) * BPC]
        in_maps.append(m)

    if TRACE:
        _ensure_ntff_hook()
    res = run_bass_kernel_spmd(nc, in_maps, list(range(NCORES)), trace=TRACE)
    LAST_EXEC_NS = res.exec_time_ns
    out = np.concatenate([res.results[c]["out"] for c in range(NCORES)], axis=0)
    return out
